# revision 20
# baseline (speedup 1.0000x reference)
"""Trainium2 Bass kernel for nn_EvolutionCrossAttention (B=4, C=128, N=32*64*64).

Strategy (8-core SPMD, N-sharded):
  The whole module reduces to, per (b, h):
     logits[n] = const + sum_c M'[b,h,c] * x[b,c,n]        (M' folds q@Wk, GN scale, rstd)
     out       = f( sum_n softmax(logits) * x[b,:,n] )      (f = small host-side matmuls)
  Softmax constants cancel between numerator and denominator, and logits have
  sigma ~= 1 after the attention scale, so exp() needs no max subtraction.
  Hence each core only computes, over its N/8 shard of bf16 x:
     pass A: per-channel mean/var partials (bn_stats) + G = mtall.T @ x  (SBUF-resident)
     (4 KB AllReduce of stats; rstd folded into R on device)
     pass B: l^T = G.T @ R ; p = exp(l^T) ; s += p.T @ x^T ; Z += p.T @ 1
  Partial (s, Z) sums add across cores on the host; all O(C^2) math is host-side.
"""
import sys

sys.path.insert(0, "/opt/trn_rl_repo")

import numpy as np
import ml_dtypes

import concourse.bass as bass
import concourse.tile as tile
from concourse import mybir
from concourse.bass_utils import run_bass_kernel_spmd

# Problem dims (hardcoded per spec)
B, C = 4, 128
N = 32 * 64 * 64          # 131072
E = 128
NH, HD = 4, 32            # heads, head dim
G, GS = 8, 16             # groupnorm groups, channels per group
EPS = 1e-5
NCORES = 8
NS = N // NCORES          # 16384 per-core columns
TILE = 512
NT = NS // TILE           # 32
CH = 128                  # transpose/matmul chunk
NCH = TILE // CH          # 4
BH = B * NH               # 16

F32 = mybir.dt.float32
BF16 = mybir.dt.bfloat16

_ISA_WAIT_LIMIT = 1


def _split_excess_waits(nc, limit=_ISA_WAIT_LIMIT):
    """This toolchain's codegen accepts only one sem wait per instruction;
    hoist extras onto same-engine nops inserted just before."""
    for bb in nc.main_func.blocks:
        insts = bb.instructions
        i = 0
        while i < len(insts):
            inst = insts[i]
            si = inst.sync_info
            if si is None or not si.on_wait or len(si.on_wait) <= limit:
                i += 1
                continue
            waits = list(si.on_wait)
            si.on_wait = waits[:limit]
            excess = waits[limit:]
            pos = i
            while excess:
                chunk, excess = excess[:limit], excess[limit:]
                nop = mybir.InstNoOp(name=nc.get_next_instruction_name(), ins=[], outs=[])
                nop.engine = inst.engine
                nop.sync_info = mybir.SyncInfo(on_wait=chunk, on_update=[])
                insts.insert(pos, nop)
                pos += 1
                i += 1
            i += 1


def _build_nc(stages=3, ncores=NCORES, waitfix=True):
    """stages: 1 = pass A + stats/R only, 2 = + logits/exp, 3 = full."""
    nc = bass.Bass()
    x = nc.declare_dram_parameter("x", [B, C, NS], BF16, isOutput=False)
    mtall = nc.declare_dram_parameter("mtall", [2, B, C, 128], BF16, isOutput=False)
    pmask = nc.declare_dram_parameter("pmask", [C, BH], F32, isOutput=False)
    gind = nc.declare_dram_parameter("gind", [C, G], F32, isOutput=False)
    sz = nc.declare_dram_parameter("sz", [B, NH, C], F32, isOutput=True)
    zout = nc.declare_dram_parameter("zvec", [1, NCH * BH], F32, isOutput=True)
    gsout = nc.declare_dram_parameter("gsums", [2 * B, G], F32, isOutput=True)

    tdma = nc.sync

    with tile.TileContext(nc) as tc:
        from contextlib import ExitStack
        with ExitStack() as ctx:
            consts = ctx.enter_context(tc.tile_pool(name="consts", bufs=1))
            small = ctx.enter_context(tc.tile_pool(name="small", bufs=1))
            xpool = ctx.enter_context(tc.tile_pool(name="xp", bufs=4))
            xtpool = ctx.enter_context(tc.tile_pool(name="xtp", bufs=8))
            ptpool = ctx.enter_context(tc.tile_pool(name="ptp", bufs=3))
            mmp = ctx.enter_context(tc.tile_pool(name="mmp", bufs=2, space="PSUM"))
            accp = ctx.enter_context(tc.tile_pool(name="accp", bufs=1, space="PSUM"))
            dram = ctx.enter_context(tc.tile_pool(name="dram", bufs=1, space="DRAM"))

            # ---- constants ----
            mtall_sb = consts.tile([C, 2, B, 128], BF16)
            nc.sync.dma_start(mtall_sb[:], mtall[:].rearrange("k b c m -> c k b m"))
            pmask_sb = consts.tile([C, BH], F32)
            nc.sync.dma_start(pmask_sb[:], pmask[:])
            gind_sb = consts.tile([C, G], F32)
            nc.sync.dma_start(gind_sb[:], gind[:])
            ones_sb = consts.tile([C, 1], BF16)
            nc.vector.memset(ones_sb[:], 1.0)

            Gsb = consts.tile([128, NS], F32, tag="Gbig")
            stat6 = consts.tile([C, B, NT, 6], F32, tag="stat6")

            # ---- pass A: stats + G ----
            for t in range(NT):
                gp = mmp.tile([128, TILE], F32, tag="mm")
                for b in range(B):
                    xt = xpool.tile([C, TILE], BF16, tag="x")
                    nc.sync.dma_start(xt[:], x[b, :, t * TILE:(t + 1) * TILE])
                    nc.vector.bn_stats(stat6[:, b, t, :], xt[:])
                    nc.tensor.matmul(gp[:], mtall_sb[:, 0, b, :], xt[:],
                                     start=(b == 0), stop=False)
                    nc.tensor.matmul(gp[:], mtall_sb[:, 1, b, :], xt[:],
                                     start=False, stop=(b == B - 1))
                nc.scalar.copy(Gsb[:, t * TILE:(t + 1) * TILE], gp[:])

            # ---- stats finish + allreduce + R ----
            mv = small.tile([C, B, 2], F32, tag="mv")
            for b in range(B):
                nc.vector.bn_aggr(mv[:, b, :], stat6[:, b, :, :])
            sq = small.tile([C, B], F32, tag="sq")
            nc.scalar.square(sq[:], mv[:, :, 0])
            ar = small.tile([C, 2 * B], F32, tag="ar")
            nc.vector.tensor_copy(ar[:, 0:B], mv[:, :, 0])
            nc.vector.tensor_add(ar[:, B:2 * B], mv[:, :, 1], sq[:])

            ar_in = dram.tile([C, 2 * B], F32, tag="arin")
            ar_out = dram.tile([C, 2 * B], F32, tag="arout")
            nc.gpsimd.dma_start(ar_in[:], ar[:])
            nc.gpsimd.collective_compute(
                "AllReduce", mybir.AluOpType.add,
                replica_groups=[list(range(ncores))],
                ins=[ar_in.opt()], outs=[ar_out.opt()],
            )
            ar2 = small.tile([C, 2 * B], F32, tag="ar2")
            nc.gpsimd.dma_start(ar2[:], ar_out[:])

            # per-(b,g) sums over the 16 channels of each group, via PE
            gsum_ps = accp.tile([2 * B, G], F32, tag="gsum")
            nc.tensor.matmul(gsum_ps[:], ar2[:], gind_sb[:], start=True, stop=True)
            gs_sb = small.tile([2 * B, G], F32, tag="gs")
            nc.vector.tensor_copy(gs_sb[:], gsum_ps[:])
            nc.gpsimd.dma_start(gsout[:], gs_sb[:])

            # rows 0..3 = sum of means per (b,g); rows 4..7 = sum of E[x^2]
            m2_sb = small.tile([B, G], F32, tag="m2")
            nc.gpsimd.dma_start(m2_sb[:], gs_sb[B:2 * B, :])
            inv = 1.0 / (GS * NCORES)
            mm2 = small.tile([B, G], F32, tag="mm2")
            nc.scalar.mul(mm2[:], m2_sb[:], inv)
            msq = small.tile([B, G], F32, tag="msq")
            nc.scalar.activation(msq[:], gs_sb[0:B, :],
                                 mybir.ActivationFunctionType.Square, scale=inv)
            var_t = small.tile([B, G], F32, tag="var")
            nc.vector.tensor_sub(var_t[:], mm2[:], msq[:])
            eps_sb = small.tile([B, 1], F32, tag="eps")
            nc.vector.memset(eps_sb[:], float(EPS))
            sdt = small.tile([B, G], F32, tag="sdt")
            nc.scalar.activation(sdt[:], var_t[:],
                                 mybir.ActivationFunctionType.Sqrt, bias=eps_sb[:])
            r_t = small.tile([B, G], F32, tag="rt")
            nc.vector.reciprocal(r_t[:], sdt[:])

            r_dram = dram.tile([B, G], F32, tag="rdram")
            nc.gpsimd.dma_start(r_dram[:], r_t[:])
            r128_dram = dram.tile([128], F32, tag="r128")
            rd = r_dram.opt()
            rd_exp = bass.AP(tensor=rd.tensor, offset=rd.offset,
                             ap=[list(rd.ap[0]), [0, NH], list(rd.ap[1])])
            nc.gpsimd.dma_start(
                r128_dram[:].rearrange("(b h g) -> b h g", b=B, h=NH), rd_exp)
            base = small.tile([128, 1], F32, tag="base")
            nc.gpsimd.dma_start(base[:], r128_dram[:, None])
            R_sb = small.tile([128, BH], F32, tag="R")
            nc.vector.tensor_scalar_mul(R_sb[:], pmask_sb[:], base[:, 0:1])

            # ---- pass B ----
            szp = [accp.tile([NH, C], F32, name=f"szp{b}", tag=f"szp{b}") for b in range(B)]
            zacc = accp.tile([1, NCH * BH], F32, tag="zacc")
            if stages >= 2:
                for t in range(NT):
                    lp = mmp.tile([128, NCH * BH], F32, tag="mm")
                    for j in range(NCH):
                        nc.tensor.matmul(
                            lp[:, j * BH:(j + 1) * BH],
                            Gsb[:, t * TILE + j * CH: t * TILE + (j + 1) * CH],
                            R_sb[:], start=True, stop=True)
                    pt = ptpool.tile([128, NCH * BH], BF16, tag="pt")
                    nc.scalar.activation(pt[:], lp[:], mybir.ActivationFunctionType.Exp)
                    if stages >= 3:
                        nc.tensor.matmul(zacc[:], ones_sb[:], pt[:],
                                         start=(t == 0), stop=(t == NT - 1))
                        for b in range(B):
                            xtb = xtpool.tile([128, NCH, CH], BF16, tag="xt")
                            tdma.dma_start(xtb[:], x[b, :, t * TILE:(t + 1) * TILE],
                                           transpose=True)
                            for j in range(NCH):
                                pslice = pt[:, j * BH + NH * b: j * BH + NH * b + NH]
                                first = (t == 0 and j == 0)
                                last = (t == NT - 1 and j == NCH - 1)
                                nc.tensor.matmul(szp[b][:], pslice, xtb[:, j, :],
                                                 start=first, stop=last)
                    else:
                        if t == 0:
                            snk = consts.tile([128, NCH * BH], F32, tag="snk")
                            nc.vector.memset(snk[:], 0.0)
                        nc.vector.tensor_add(snk[:], snk[:], pt[:])

            if stages >= 3:
                for b in range(B):
                    ssb = small.tile([NH, C], F32, name=f"ssb{b}", tag=f"ssb{b}")
                    nc.vector.tensor_copy(ssb[:], szp[b][:])
                    nc.gpsimd.dma_start(sz[b], ssb[:])
                zsb = small.tile([1, NCH * BH], F32, tag="zsb")
                nc.vector.tensor_copy(zsb[:], zacc[:])
                nc.gpsimd.dma_start(zout[:], zsb[:])
            else:
                zb = small.tile([B, NH * C], F32, tag="zb")
                nc.vector.memset(zb[:], 0.0)
                if stages == 2:
                    nc.vector.tensor_copy(zb[0:B, 0:NCH * BH].rearrange("a b -> a b"),
                                          snk[0:B, :])
                nc.gpsimd.dma_start(sz[:].rearrange("b h c -> b (h c)"), zb[:])
                zb2 = small.tile([1, NCH * BH], F32, tag="zb2")
                nc.vector.memset(zb2[:], 0.0)
                nc.gpsimd.dma_start(zout[:], zb2[:])

    if waitfix:
        _split_excess_waits(nc)
    return nc


_NC_CACHE = {}


def _get_nc():
    if "nc" not in _NC_CACHE:
        _NC_CACHE["nc"] = _build_nc()
    return _NC_CACHE["nc"]


def _host_prep(diff_spatial, evolution_feat, ln_g, ln_b, gn_g, Wq, bq, Wk, bk):
    """Everything O(C^2): layernorm, q, fold q@Wk with GN affine + attn scale."""
    e = evolution_feat.astype(np.float64)
    mu = e.mean(axis=-1, keepdims=True)
    var = e.var(axis=-1, keepdims=True)
    e = (e - mu) / np.sqrt(var + EPS) * ln_g.astype(np.float64) + ln_b.astype(np.float64)
    q = e @ Wq.T.astype(np.float64) + bq.astype(np.float64)      # (B, C)
    q = q.reshape(B, NH, HD)
    # M[b,h,c] = sum_d q[b,h,d] Wk[h*HD+d, c]
    Wkr = Wk.astype(np.float64).reshape(NH, HD, C)
    M = np.einsum("bhd,hdc->bhc", q, Wkr)
    Mfold = M * gn_g.astype(np.float64)[None, None, :] * (HD ** -0.5)

    cg = np.arange(C) // GS                                       # channel -> group
    # mtall[b, c, p] for p = b'*32 + h*8 + g, masked to b'==b and g==cg[c]
    mtall = np.zeros((B, C, 128), np.float64)
    for b in range(B):
        for h in range(NH):
            for g in range(G):
                p = b * 32 + h * 8 + g
                sel = cg == g
                mtall[b, sel, p] = Mfold[b, h, sel]
    # hi/lo bf16 split: two accumulating matmuls recover ~16 mantissa bits
    mt_hi = mtall.astype(ml_dtypes.bfloat16)
    mt_lo = (mtall - mt_hi.astype(np.float64)).astype(ml_dtypes.bfloat16)
    mt2 = np.stack([mt_hi, mt_lo], axis=0)                        # (2, B, C, 128)
    pmask = np.zeros((128, BH), np.float32)
    for b in range(B):
        for h in range(NH):
            for g in range(G):
                pmask[b * 32 + h * 8 + g, b * NH + h] = 1.0
    gindm = (cg[:, None] == np.arange(G)[None, :]).astype(np.float32)
    return q, mt2, pmask, gindm


def kernel(diff_spatial, evolution_feat, ln_g, ln_b, gn_g, gn_b,
           Wq, bq, Wk, bk, Wv, bv, Wo, bo):
    nc = _get_nc()
    xfull = np.asarray(diff_spatial, np.float32).reshape(B, C, N)
    x_bf = xfull.astype(ml_dtypes.bfloat16)

    q, mtall, pmask, gindm = _host_prep(
        np.asarray(diff_spatial, np.float32), np.asarray(evolution_feat, np.float32),
        np.asarray(ln_g, np.float32), np.asarray(ln_b, np.float32),
        np.asarray(gn_g, np.float32), np.asarray(Wq, np.float32),
        np.asarray(bq, np.float32), np.asarray(Wk, np.float32),
        np.asarray(bk, np.float32))

    in_maps = []
    for i in range(NCORES):
        in_maps.append({
            "x": np.ascontiguousarray(x_bf[:, :, i * NS:(i + 1) * NS]),
            "mtall": mtall,
            "pmask": pmask,
            "gind": gindm,
        })
    res = run_bass_kernel_spmd(nc, in_maps, list(range(NCORES)))
    return _host_finish(res.results, gn_g, gn_b, Wv, bv, Wo, bo)


def _host_finish(results, gn_g, gn_b, Wv, bv, Wo, bo):
    gs = results[0]["gsums"].astype(np.float64)                 # (2B, G)
    mean_g = gs[0:B, :] / (GS * NCORES)                          # (B, G)
    ex2_g = gs[B:2 * B, :] / (GS * NCORES)
    var_g = ex2_g - mean_g ** 2
    r_g = 1.0 / np.sqrt(var_g + EPS)

    s_tot = np.zeros((B, NH, C), np.float64)
    z_tot = np.zeros((B, NH), np.float64)
    for r in results:
        s_tot += r["sz"].astype(np.float64)                     # (B, NH, C)
        zv = r["zvec"].astype(np.float64).reshape(NCH, B, NH)    # (j, b, h)
        z_tot += zv.sum(axis=0)

    cg = np.arange(C) // GS
    a = r_g[:, cg] * np.asarray(gn_g, np.float64)[None, :]       # (B, C)
    d = np.asarray(gn_b, np.float64)[None, :] - mean_g[:, cg] * a
    y = a[:, None, :] * (s_tot / z_tot[:, :, None]) + d[:, None, :]   # (B, NH, C)

    Wvr = np.asarray(Wv, np.float64).reshape(NH, HD, C)
    o1 = np.einsum("hdc,bhc->bhd", Wvr, y).reshape(B, C) + np.asarray(bv, np.float64)
    out = o1 @ np.asarray(Wo, np.float64).T + np.asarray(bo, np.float64)
    return out.astype(np.float32)


# revision 40
# speedup vs baseline: 603.8069x; 603.8069x over previous
"""Trainium2 Bass kernel for nn_EvolutionCrossAttention (B=4, C=128, N=32*64*64).

Strategy (8-core SPMD, N-sharded):
  The whole module reduces to, per (b, h):
     logits[n] = const + sum_c M'[b,h,c] * x[b,c,n]        (M' folds q@Wk, GN scale, rstd)
     out       = f( sum_n softmax(logits) * x[b,:,n] )      (f = small host-side matmuls)
  Softmax constants cancel between numerator and denominator, and logits have
  sigma ~= 1 after the attention scale, so exp() needs no max subtraction.
  Hence each core only computes, over its N/8 shard of bf16 x:
     pass A: per-channel mean/var partials (bn_stats) + G = mtall.T @ x  (SBUF-resident)
     (4 KB AllReduce of stats; rstd folded into R on device)
     pass B: l^T = G.T @ R ; p = exp(l^T) ; s += p.T @ x^T ; Z += p.T @ 1
  Partial (s, Z) sums add across cores on the host; all O(C^2) math is host-side.
"""
import sys

sys.path.insert(0, "/opt/trn_rl_repo")

import numpy as np
import ml_dtypes

import concourse.bass as bass
import concourse.tile as tile
from concourse import mybir
from concourse.bass_utils import run_bass_kernel_spmd

# Problem dims (hardcoded per spec)
B, C = 4, 128
N = 32 * 64 * 64          # 131072
E = 128
NH, HD = 4, 32            # heads, head dim
G, GS = 8, 16             # groupnorm groups, channels per group
EPS = 1e-5
NCORES = 8
NS = N // NCORES          # 16384 per-core columns
TILE = 512
NT = NS // TILE           # 32
CH = 128                  # transpose/matmul chunk
NCH = TILE // CH          # 4
BH = B * NH               # 16

F32 = mybir.dt.float32
BF16 = mybir.dt.bfloat16

_ISA_WAIT_LIMIT = 1


def _split_excess_waits(nc, limit=_ISA_WAIT_LIMIT):
    """This toolchain's codegen accepts only one sem wait per instruction;
    hoist extras onto same-engine nops inserted just before."""
    for bb in nc.main_func.blocks:
        insts = bb.instructions
        i = 0
        while i < len(insts):
            inst = insts[i]
            si = inst.sync_info
            if si is None or not si.on_wait or len(si.on_wait) <= limit:
                i += 1
                continue
            waits = list(si.on_wait)
            si.on_wait = waits[:limit]
            excess = waits[limit:]
            pos = i
            while excess:
                chunk, excess = excess[:limit], excess[limit:]
                nop = mybir.InstNoOp(name=nc.get_next_instruction_name(), ins=[], outs=[])
                nop.engine = inst.engine
                nop.sync_info = mybir.SyncInfo(on_wait=chunk, on_update=[])
                insts.insert(pos, nop)
                pos += 1
                i += 1
            i += 1


def _build_nc(stages=3, ncores=NCORES, waitfix=True):
    """v2: x stays SBUF-resident; PE transposes; single DMA pass."""
    nc = bass.Bass()
    x = nc.declare_dram_parameter("x", [B, C, NS], BF16, isOutput=False)
    mtall = nc.declare_dram_parameter("mtall", [2, B, C, 128], BF16, isOutput=False)
    pmask = nc.declare_dram_parameter("pmask", [C, BH], F32, isOutput=False)
    gind = nc.declare_dram_parameter("gind", [C, G], F32, isOutput=False)
    ident = nc.declare_dram_parameter("ident", [C, C], BF16, isOutput=False)
    sz = nc.declare_dram_parameter("sz", [B, NH, C + 1], F32, isOutput=True)
    gsout = nc.declare_dram_parameter("gsums", [2 * B, G], F32, isOutput=True)

    FP16 = mybir.dt.float16
    QT = NT // 4                     # tiles per x quarter-part

    with tile.TileContext(nc) as tc:
        from contextlib import ExitStack
        with ExitStack() as ctx:
            consts = ctx.enter_context(tc.tile_pool(name="consts", bufs=1))
            small = ctx.enter_context(tc.tile_pool(name="small", bufs=1))
            xtspool = ctx.enter_context(tc.tile_pool(name="xts", bufs=5))
            ptpool = ctx.enter_context(tc.tile_pool(name="ptp", bufs=3))
            mmp = ctx.enter_context(tc.tile_pool(name="mmp", bufs=2, space="PSUM"))
            xtpp = ctx.enter_context(tc.tile_pool(name="xtpp", bufs=2, space="PSUM"))
            accp = ctx.enter_context(tc.tile_pool(name="accp", bufs=1, space="PSUM"))
            dram = ctx.enter_context(tc.tile_pool(name="dram", bufs=1, space="DRAM"))

            # ---- constants ----
            mtall_sb = consts.tile([C, 2, B, 128], BF16)
            nc.sync.dma_start(mtall_sb[:], mtall[:].rearrange("k b c m -> c k b m"))
            pmask_sb = consts.tile([C, BH], F32)
            nc.sync.dma_start(pmask_sb[:], pmask[:])
            gind_sb = consts.tile([C, G], F32)
            nc.sync.dma_start(gind_sb[:], gind[:])
            ident_sb = consts.tile([C, C], BF16)
            nc.sync.dma_start(ident_sb[:], ident[:])
            ones_f32 = consts.tile([C, B * NCH], BF16, tag="ones4")
            nc.vector.memset(ones_f32[:], 1.0)

            # x resident in SBUF: 16 quarter-batch parts for load/compute overlap
            xparts = [[None] * 4 for _ in range(B)]
            for qq in range(4):
                for b in range(B):
                    xp = consts.tile([C, QT, TILE], BF16, name=f"xsb{b}_{qq}",
                                     tag=f"xsb{b}_{qq}")
                    nc.sync.dma_start(
                        xp[:], x[b, :, qq * QT * TILE:(qq + 1) * QT * TILE])
                    xparts[b][qq] = xp

            def xsl(b, t, lo, hi):
                return xparts[b][t // QT][:, t % QT, lo:hi]

            Gsb = consts.tile([128, NS], FP16, tag="Gbig")
            stat6 = consts.tile([C, B, NT, 6], F32, tag="stat6")

            xts_map = {}

            def emit_transpose(t):
                xts = xtspool.tile([128, B * NCH, CH + 1], BF16, tag="xts")
                for half in range(2):
                    xtp = xtpp.tile([128, 2 * NCH, CH], BF16, tag="xtp")
                    for bb in range(2):
                        b = half * 2 + bb
                        for j in range(NCH):
                            nc.tensor.transpose(xtp[:, bb * NCH + j, :],
                                                xsl(b, t, j * CH, (j + 1) * CH),
                                                ident_sb[:])
                    nc.scalar.copy(
                        xts[:, half * 2 * NCH:(half + 1) * 2 * NCH, 0:CH], xtp[:])
                nc.scalar.mul(xts[:, :, CH:CH + 1], ones_f32[:, 0:B * NCH, None], 1.0)
                xts_map[t] = xts

            # ---- pass A: stats + G ----
            for t in range(NT):
                gp = mmp.tile([128, TILE], F32, tag="mm")
                for b in range(B):
                    nc.vector.bn_stats(stat6[:, b, t, :], xsl(b, t, 0, TILE))
                    nc.tensor.matmul(gp[:], mtall_sb[:, 0, b, :], xsl(b, t, 0, TILE),
                                     start=(b == 0), stop=False)
                    nc.tensor.matmul(gp[:], mtall_sb[:, 1, b, :], xsl(b, t, 0, TILE),
                                     start=False, stop=(b == B - 1))
                nc.scalar.copy(Gsb[:, t * TILE:(t + 1) * TILE], gp[:])

            # Pre-emit the first few tiles' transposes so the scheduler can run
            # them on PE while the stats collective is in flight.
            PREK = 5
            for t in range(PREK):
                emit_transpose(t)

            # ---- stats finish + allreduce + R ----
            mv = small.tile([C, B, 2], F32, tag="mv")
            for b in range(B):
                nc.vector.bn_aggr(mv[:, b, :], stat6[:, b, :, :])
            sq = small.tile([C, B], F32, tag="sq")
            nc.scalar.square(sq[:], mv[:, :, 0])
            ar = small.tile([C, 2 * B], F32, tag="ar")
            nc.vector.tensor_copy(ar[:, 0:B], mv[:, :, 0])
            nc.vector.tensor_add(ar[:, B:2 * B], mv[:, :, 1], sq[:])

            ar_in = dram.tile([C, 2 * B], F32, tag="arin")
            ar_out = dram.tile([C, 2 * B], F32, tag="arout")
            nc.gpsimd.dma_start(ar_in[:], ar[:])
            nc.gpsimd.collective_compute(
                "AllReduce", mybir.AluOpType.add,
                replica_groups=[list(range(ncores))],
                ins=[ar_in.opt()], outs=[ar_out.opt()],
            )
            ar2 = small.tile([C, 2 * B], F32, tag="ar2")
            nc.gpsimd.dma_start(ar2[:], ar_out[:])

            # per-(b,g) sums over the 16 channels of each group, via PE
            gsum_ps = mmp.tile([2 * B, G], F32, tag="mm")
            nc.tensor.matmul(gsum_ps[:], ar2[:], gind_sb[:], start=True, stop=True)
            gs_sb = small.tile([2 * B, G], F32, tag="gs")
            nc.vector.tensor_copy(gs_sb[:], gsum_ps[:])
            nc.gpsimd.dma_start(gsout[:], gs_sb[:])

            # rows 0..3 = sum of means per (b,g); rows 4..7 = sum of E[x^2]
            m2_sb = small.tile([B, G], F32, tag="m2")
            nc.gpsimd.dma_start(m2_sb[:], gs_sb[B:2 * B, :])
            inv = 1.0 / (GS * NCORES)
            mm2 = small.tile([B, G], F32, tag="mm2")
            nc.scalar.mul(mm2[:], m2_sb[:], inv)
            msq = small.tile([B, G], F32, tag="msq")
            nc.scalar.activation(msq[:], gs_sb[0:B, :],
                                 mybir.ActivationFunctionType.Square, scale=inv)
            var_t = small.tile([B, G], F32, tag="var")
            nc.vector.tensor_sub(var_t[:], mm2[:], msq[:])
            eps_sb = small.tile([B, 1], F32, tag="eps")
            nc.vector.memset(eps_sb[:], float(EPS))
            sdt = small.tile([B, G], F32, tag="sdt")
            nc.scalar.activation(sdt[:], var_t[:],
                                 mybir.ActivationFunctionType.Sqrt, bias=eps_sb[:])
            r_t = small.tile([B, G], F32, tag="rt")
            nc.vector.reciprocal(r_t[:], sdt[:])

            r_dram = dram.tile([B, G], F32, tag="rdram")
            nc.gpsimd.dma_start(r_dram[:], r_t[:])
            r128_dram = dram.tile([128], F32, tag="r128")
            rd = r_dram.opt()
            rd_exp = bass.AP(tensor=rd.tensor, offset=rd.offset,
                             ap=[list(rd.ap[0]), [0, NH], list(rd.ap[1])])
            nc.gpsimd.dma_start(
                r128_dram[:].rearrange("(b h g) -> b h g", b=B, h=NH), rd_exp)
            base = small.tile([128, 1], F32, tag="base")
            nc.gpsimd.dma_start(base[:], r128_dram[:, None])
            R_sb = small.tile([128, BH], F32, tag="R")
            nc.vector.tensor_scalar_mul(R_sb[:], pmask_sb[:], base[:, 0:1])
            # fp16 hi/lo split of R (rhs dtype must match fp16 Gsb)
            R_hi = small.tile([128, BH], FP16, tag="Rhi")
            nc.vector.tensor_copy(R_hi[:], R_sb[:])
            R_lo = small.tile([128, BH], FP16, tag="Rlo")
            nc.vector.tensor_sub(R_lo[:], R_sb[:], R_hi[:])

            # ---- pass B ----
            szp = [accp.tile([NH, C + 1], F32, name=f"szp{b}", tag=f"szp{b}")
                   for b in range(B)]
            for t in range(NT):
                lp = mmp.tile([128, NCH * BH], F32, tag="mm")
                for j in range(NCH):
                    gsl = Gsb[:, t * TILE + j * CH: t * TILE + (j + 1) * CH]
                    nc.tensor.matmul(lp[:, j * BH:(j + 1) * BH], gsl, R_hi[:],
                                     start=True, stop=False)
                    nc.tensor.matmul(lp[:, j * BH:(j + 1) * BH], gsl, R_lo[:],
                                     start=False, stop=True)
                pt = ptpool.tile([128, NCH * BH], BF16, tag="pt")
                nc.scalar.activation(pt[:], lp[:], mybir.ActivationFunctionType.Exp)
                if t not in xts_map:
                    emit_transpose(t)
                xts = xts_map.pop(t)
                for b in range(B):
                    for j in range(NCH):
                        pslice = pt[:, j * BH + NH * b: j * BH + NH * b + NH]
                        first = (t == 0 and j == 0)
                        last = (t == NT - 1 and j == NCH - 1)
                        nc.tensor.matmul(szp[b][:], pslice,
                                         xts[:, b * NCH + j, :],
                                         start=first, stop=last)

            for b in range(B):
                ssb = small.tile([NH, C + 1], F32, name=f"ssb{b}", tag=f"ssb{b}")
                nc.vector.tensor_copy(ssb[:], szp[b][:])
                nc.gpsimd.dma_start(sz[b], ssb[:])

    if waitfix:
        _split_excess_waits(nc)
    return nc


_NC_CACHE = {}


def _get_nc():
    if "nc" not in _NC_CACHE:
        _NC_CACHE["nc"] = _build_nc()
    return _NC_CACHE["nc"]


def _host_prep(diff_spatial, evolution_feat, ln_g, ln_b, gn_g, Wq, bq, Wk, bk):
    """Everything O(C^2): layernorm, q, fold q@Wk with GN affine + attn scale."""
    e = evolution_feat.astype(np.float64)
    mu = e.mean(axis=-1, keepdims=True)
    var = e.var(axis=-1, keepdims=True)
    e = (e - mu) / np.sqrt(var + EPS) * ln_g.astype(np.float64) + ln_b.astype(np.float64)
    q = e @ Wq.T.astype(np.float64) + bq.astype(np.float64)      # (B, C)
    q = q.reshape(B, NH, HD)
    # M[b,h,c] = sum_d q[b,h,d] Wk[h*HD+d, c]
    Wkr = Wk.astype(np.float64).reshape(NH, HD, C)
    M = np.einsum("bhd,hdc->bhc", q, Wkr)
    Mfold = M * gn_g.astype(np.float64)[None, None, :] * (HD ** -0.5)

    cg = np.arange(C) // GS                                       # channel -> group
    # mtall[b, c, p] for p = b'*32 + h*8 + g, masked to b'==b and g==cg[c]
    mtall = np.zeros((B, C, 128), np.float64)
    for b in range(B):
        for h in range(NH):
            for g in range(G):
                p = b * 32 + h * 8 + g
                sel = cg == g
                mtall[b, sel, p] = Mfold[b, h, sel]
    # hi/lo bf16 split: two accumulating matmuls recover ~16 mantissa bits
    mt_hi = mtall.astype(ml_dtypes.bfloat16)
    mt_lo = (mtall - mt_hi.astype(np.float64)).astype(ml_dtypes.bfloat16)
    mt2 = np.stack([mt_hi, mt_lo], axis=0)                        # (2, B, C, 128)
    pmask = np.zeros((128, BH), np.float32)
    for b in range(B):
        for h in range(NH):
            for g in range(G):
                pmask[b * 32 + h * 8 + g, b * NH + h] = 1.0
    gindm = (cg[:, None] == np.arange(G)[None, :]).astype(np.float32)
    return q, mt2, pmask, gindm


def kernel(diff_spatial, evolution_feat, ln_g, ln_b, gn_g, gn_b,
           Wq, bq, Wk, bk, Wv, bv, Wo, bo):
    nc = _get_nc()
    xfull = np.asarray(diff_spatial, np.float32).reshape(B, C, N)
    x_bf = xfull.astype(ml_dtypes.bfloat16)

    q, mtall, pmask, gindm = _host_prep(
        np.asarray(diff_spatial, np.float32), np.asarray(evolution_feat, np.float32),
        np.asarray(ln_g, np.float32), np.asarray(ln_b, np.float32),
        np.asarray(gn_g, np.float32), np.asarray(Wq, np.float32),
        np.asarray(bq, np.float32), np.asarray(Wk, np.float32),
        np.asarray(bk, np.float32))

    identv = np.eye(C, dtype=np.float32).astype(ml_dtypes.bfloat16)
    in_maps = []
    for i in range(NCORES):
        in_maps.append({
            "x": np.ascontiguousarray(x_bf[:, :, i * NS:(i + 1) * NS]),
            "mtall": mtall,
            "pmask": pmask,
            "gind": gindm,
            "ident": identv,
        })
    res = run_bass_kernel_spmd(nc, in_maps, list(range(NCORES)))
    return _host_finish(res.results, gn_g, gn_b, Wv, bv, Wo, bo)


def _host_finish(results, gn_g, gn_b, Wv, bv, Wo, bo):
    gs = results[0]["gsums"].astype(np.float64)                 # (2B, G)
    mean_g = gs[0:B, :] / (GS * NCORES)                          # (B, G)
    ex2_g = gs[B:2 * B, :] / (GS * NCORES)
    var_g = ex2_g - mean_g ** 2
    r_g = 1.0 / np.sqrt(var_g + EPS)

    s_tot = np.zeros((B, NH, C), np.float64)
    z_tot = np.zeros((B, NH), np.float64)
    for r in results:
        szv = r["sz"].astype(np.float64)                        # (B, NH, C+1)
        s_tot += szv[:, :, 0:C]
        z_tot += szv[:, :, C]

    cg = np.arange(C) // GS
    a = r_g[:, cg] * np.asarray(gn_g, np.float64)[None, :]       # (B, C)
    d = np.asarray(gn_b, np.float64)[None, :] - mean_g[:, cg] * a
    y = a[:, None, :] * (s_tot / z_tot[:, :, None]) + d[:, None, :]   # (B, NH, C)

    Wvr = np.asarray(Wv, np.float64).reshape(NH, HD, C)
    o1 = np.einsum("hdc,bhc->bhd", Wvr, y).reshape(B, C) + np.asarray(bv, np.float64)
    out = o1 @ np.asarray(Wo, np.float64).T + np.asarray(bo, np.float64)
    return out.astype(np.float32)
